# revision 51
# baseline (speedup 1.0000x reference)
"""Multi-head attention (B=2, S=2048, E=1024, H=16, causal) on 8 TRN2 cores.

Sharding: core c -> batch b = c//4, head group g = c%4 (4 heads each).
Each core computes QKV projection for its heads, causal flash-style
attention (no-max softmax, denominator via ones-column appended to V),
and a partial output projection against a 256-row slice of W_proj.
Host sums the 4 partial projections per batch (the "all-reduce") and
stacks the 2 batches.

All matmul operands are bf16 (fp32 accumulation in PSUM); fp32r HIGH
mode triggered sustained power throttling (~60% effective PE clock),
bf16 halves PE+SBUF power and DMA bytes. Activation layouts avoid all
on-device transposes: the host passes x[b].T per core.
"""
import sys

sys.path.insert(0, "/opt/trn_rl_repo")

import numpy as np
import ml_dtypes

import concourse.bacc as bacc
import concourse.mybir as mybir
from concourse import tile
from concourse.bass_utils import run_bass_kernel_spmd

B, S, E, H, D = 2, 2048, 1024, 16, 64
SCALE = D ** -0.5
N_CORES = 8
HL = 4            # heads per core
GC = 256          # channel columns per core (HL * D)
F32 = mybir.dt.float32
F32R = mybir.dt.float32r
BF16 = mybir.dt.bfloat16

_CACHED_NC = None


def _build():
    nc = bacc.Bacc("TRN2", target_bir_lowering=False, debug=False,
                   num_devices=N_CORES)

    xT = nc.dram_tensor("xT", [E, S], BF16, kind="ExternalInput")
    w = nc.dram_tensor("w", [E, 3 * GC], BF16, kind="ExternalInput")
    wp = nc.dram_tensor("wp", [GC, E], BF16, kind="ExternalInput")
    mask = nc.dram_tensor("mask", [128, 128], BF16, kind="ExternalInput")
    y = nc.dram_tensor("y", [S, E], BF16, kind="ExternalOutput")

    ET = E // 128     # 8 e-tiles
    NS = S // 512     # 4 s-chunks of 512
    NT = S // 128     # 16 s-tiles of 128

    with tile.TileContext(nc) as tc:
        with (
            tc.tile_pool(name="const", bufs=1) as cst,
            tc.tile_pool(name="acts", bufs=1) as acts,
            tc.tile_pool(name="expp", bufs=5) as expp,
            tc.tile_pool(name="small", bufs=2) as small,
            tc.tile_pool(name="ysb", bufs=3) as ysbp,
            tc.tile_pool(name="psS", bufs=2, space="PSUM") as psS,
            tc.tile_pool(name="psO", bufs=4, space="PSUM") as psO,
        ):
            # ---- constant loads -------------------------------------------
            xt = cst.tile([128, ET, S], BF16)          # x[b]^T  (e on partitions)
            wt = cst.tile([128, ET, 3 * GC], BF16)     # W_qkv slice (e on partitions)
            wpt = cst.tile([128, 2, E], BF16)          # W_proj slice (c on partitions)
            mk = cst.tile([128, 128], BF16)            # tril(128) causal mask

            # x streams on the sync DGE queue while weights load in parallel
            # on the scalar (ACT) DGE queue — the only two HWDGE queues; ACT's
            # instruction queue is idle during the head so submissions are
            # free. Slices match the first QKV et-chain's consumption order
            # (q jt0 cols + k jt0 cols first, x chunk 0 in 2-et pieces).
            wr = w[:].rearrange("(t p) j -> p t j", p=128)
            xTr = xT[:].rearrange("(t p) s -> p t s", p=128)
            nc.scalar.dma_start(wt[:, :, 0:128], wr[:, :, 0:128])
            nc.sync.dma_start(xt[:, 0:2, 0:512], xTr[:, 0:2, 0:512])
            nc.scalar.dma_start(wt[:, :, 256:384], wr[:, :, 256:384])
            nc.sync.dma_start(xt[:, 2:4, 0:512], xTr[:, 2:4, 0:512])
            nc.scalar.dma_start(wt[:, :, 128:256], wr[:, :, 128:256])
            nc.sync.dma_start(xt[:, 4:6, 0:512], xTr[:, 4:6, 0:512])
            nc.scalar.dma_start(wt[:, :, 384:512], wr[:, :, 384:512])
            nc.sync.dma_start(xt[:, 6:8, 0:512], xTr[:, 6:8, 0:512])
            nc.scalar.dma_start(mk[:], mask[:])
            nc.scalar.dma_start(wt[:, :, 512:768], wr[:, :, 512:768])
            nc.sync.dma_start(xt[:, :, 512:1024], xTr[:, :, 512:1024])
            nc.scalar.dma_start(wpt[:], wp[:].rearrange("(t p) e -> p t e", p=128))
            for sc in range(2, NS):
                nc.sync.dma_start(
                    xt[:, :, 512 * sc:512 * (sc + 1)], xTr[:, :, 512 * sc:512 * (sc + 1)]
                )

            # ---- activation buffers ---------------------------------------
            # qt/kt: [pair, j(128 part: head 2p on 0-63, head 2p+1 on 64-127), s]
            qt = acts.tile([128, 2, S], BF16)
            kt = acts.tile([128, 2, S], BF16)
            # v_aug: per s-tile, per head 65 cols (64 data + ones)
            vt = acts.tile([128, NT, HL * 65], BF16)
            # attention output^T, proj lhsT layout: c on partitions.
            # One tile per (s-chunk, pair) so proj's reads only depend on
            # the norms that actually produced that chunk/pair (the tile
            # dep tracker is conservative across subtile writes).
            otc = [[acts.tile([128, 512], BF16, name=f"ot{j}_{p}")
                    for p in range(2)] for j in range(NS)]

            # small dedicated warm-up source so warm matmuls don't wait on
            # the big vt memset
            wsrc = cst.tile([128, 640], BF16)
            nc.vector.memset(wsrc[:], 0.5)
            ones64 = cst.tile([128, 64], BF16)
            nc.vector.memset(ones64[:], 1.0)
            warm = cst.tile([128, 16], F32)
            nc.scalar.activation(warm[:], wsrc[:, 0:16],
                                 mybir.ActivationFunctionType.Exp)
            # dummy matmuls during the DMA head keep the PE HAM-warm so QKV
            # starts at full clock
            for wi in range(12):
                pw = psS.tile([128, 1024], F32, tag="ps", name="pw")
                nc.tensor.matmul(pw[:, 0:512], wsrc[:, 0:128], wsrc[:, 128:640],
                                 start=True, stop=True)
            nc.vector.memset(vt[:], 1.0)

            # ---- QKV projection ------------------------------------------
            def qk_half(sc, jt, which):
                s0 = 512 * sc
                dest = qt if which == 0 else kt
                ps = psO.tile([128, 512], F32, tag="po", bufs=2, name="psqk")
                for et in range(ET):
                    nc.tensor.matmul(
                        ps[:, 0:512],
                        wt[:, et, 256 * which + 128 * jt:256 * which + 128 * (jt + 1)],
                        xt[:, et, s0:s0 + 512],
                        start=(et == 0),
                        stop=(et == ET - 1),
                    )
                nc.vector.tensor_copy(dest[:, jt, s0:s0 + 512], ps[:, 0:512])

            def qk_pair(sc, jt):
                qk_half(sc, jt, 0)
                qk_half(sc, jt, 1)

            def v_st(sc, st4):
                st = 4 * sc + st4
                ps = psO.tile([128, 512], F32, tag="po", bufs=2, name="psv")
                for et in range(ET):
                    nc.tensor.matmul(
                        ps[:, 0:256],
                        xt[:, et, 128 * st:128 * (st + 1)],
                        wt[:, et, 512:768],
                        start=(et == 0),
                        stop=(et == ET - 1),
                    )
                nc.vector.tensor_copy(
                    vt[:, st].rearrange("p (h m) -> p h m", h=HL)[:, :, 0:64],
                    ps[:, 0:256].rearrange("p (h m) -> p h m", h=HL),
                )

            def v_chunk(sc, lo=0, hi=4):
                for st4 in range(lo, hi):
                    v_st(sc, st4)

            # ---- attention for one (pair, q-chunk) ------------------------
            def attn_part(pr, jq, ik_lo, ik_hi, o_ab, last_fill=None):
                s0 = 512 * jq
                nik = 4 * jq + 4
                for ik in range(ik_lo, ik_hi):
                    t = ik - 4 * jq
                    c0 = 128 * t if t > 0 else 0   # exact-causal column trim
                    ps = psS.tile([128, 1024], F32)
                    for ab in range(2):
                        p0 = 64 * ab
                        nc.tensor.matmul(
                            ps[:, 512 * ab + c0:512 * (ab + 1)],
                            kt[p0:p0 + 64, pr, 128 * ik:128 * (ik + 1)],
                            qt[p0:p0 + 64, pr, s0 + c0:s0 + 512],
                            start=True,
                            stop=True,
                            tile_position=(p0, 0),
                        )
                    e = expp.tile([128, 1024], BF16, tag="exps", name="exps")
                    e3 = e[:].rearrange("p (h n) -> p h n", h=2)[:, :, c0:512]
                    ps3 = ps[:].rearrange("p (h n) -> p h n", h=2)[:, :, c0:512]
                    nc.scalar.activation(e3, ps3, mybir.ActivationFunctionType.Exp,
                                         scale=float(SCALE))
                    if t >= 0:
                        # only the first 128 cols of the trimmed range are
                        # partially masked; the rest is fully unmasked
                        for ab in range(2):
                            nc.vector.tensor_mul(
                                e[:, 512 * ab + c0:512 * ab + c0 + 128],
                                e[:, 512 * ab + c0:512 * ab + c0 + 128],
                                mk[:],
                            )
                    if ik == nik - 1 and last_fill is not None:
                        # PE filler between the final scores/exp and final AV
                        # so the PE doesn't drain while ACT finishes the exp
                        last_fill()
                    for ab in range(2):
                        h = 2 * pr + ab
                        nc.tensor.matmul(
                            o_ab[ab][0:65, c0:512],
                            vt[:, ik, 65 * h:65 * h + 65],
                            e[:, 512 * ab + c0:512 * (ab + 1)],
                            start=(ik == 0),
                            stop=(ik == nik - 1),
                            skip_group_check=True,
                        )
            # normalize: out^T[d, s] = o[d, s] * (1 / o[64, s]).
            # Engines can shift partitions between in and out APs, so head
            # ab=1 writes ot partitions 64-127 directly.
            def attn_norm_tail_pre(o_ab):
                """copies + reciprocals for both heads (no PE work)"""
                pre = []
                for ab in range(2):
                    o = o_ab[ab]
                    osb = small.tile([128, 512], F32, tag="osb", name="osb")
                    nc.scalar.copy(osb[0:65, :], o[0:65, :])
                    rz = small.tile([128, 512], F32, tag="rz", name="rz")
                    nc.vector.tensor_copy(rz[0:1, :], o[64:65, :])
                    rr = small.tile([128, 512], F32, tag="rr", name="rr")
                    nc.vector.reciprocal_approx_fast(rr[0:1, :], rz[0:1, :])
                    # bf16 copy so the PE broadcast runs at 1 cyc/row
                    rh = small.tile([128, 512], BF16, tag="rh", name="rh")
                    nc.vector.tensor_copy(rh[0:1, :], rr[0:1, :])
                    pre.append((osb, rh))
                return pre

            def attn_norm_tail_post(pr, jq, pre):
                """PE broadcast + final muls; PE is otherwise idle at the end"""
                for ab in range(2):
                    osb, rh = pre[ab]
                    pb = psO.tile([128, 512], F32, tag="oab", bufs=2,
                                  name="pb")
                    nc.tensor.matmul(pb[0:64, :], ones64[0:1, 0:64],
                                     rh[0:1, :],
                                     start=True, stop=True)
                    nc.vector.tensor_mul(
                        otc[jq][pr][64 * ab:64 * ab + 64, :],
                        osb[0:64, :], pb[0:64, :])

            def attn_norm(pr, jq, o_ab):
                for ab in range(2):
                    o = o_ab[ab]
                    osb = small.tile([128, 512], F32, tag="osb", name="osb")
                    nc.vector.tensor_copy(osb[0:65, :], o[0:65, :])
                    # r row to physical partition 0 (partition_broadcast needs
                    # it); reciprocal before broadcast so only [1,512] work
                    rz = small.tile([128, 512], F32, tag="rz", name="rz")
                    nc.vector.tensor_copy(rz[0:1, :], o[64:65, :])
                    rr = small.tile([128, 512], F32, tag="rr", name="rr")
                    nc.vector.reciprocal_approx_fast(rr[0:1, :], rz[0:1, :])
                    rb = small.tile([128, 512], F32, tag="rb", name="rb")
                    nc.gpsimd.partition_broadcast(rb[0:64, :], rr[0:1, :])
                    # mul on gpsimd: on vector it delays the next phase's
                    # mask-muls, which gate AV matmuls behind exp
                    nc.gpsimd.tensor_mul(
                        otc[jq][pr][64 * ab:64 * ab + 64, :],
                        osb[0:64, :], rb[0:64, :])

            # ---- output projection for one s-tile --------------------------
            def proj_st(jq, st4, copy_eng=None):
                st = 4 * jq + st4
                for nk in range(2):
                    py = psO.tile([128, 512], F32, tag="po", bufs=2, name="py")
                    for ct in range(2):
                        nc.tensor.matmul(
                            py[:],
                            otc[jq][ct][:, 128 * st4:128 * (st4 + 1)],
                            wpt[:, ct, 512 * nk:512 * (nk + 1)],
                            start=(ct == 0),
                            stop=(ct == 1),
                        )
                    ys = ysbp.tile([128, 512], BF16)
                    if copy_eng == "alt" and nk == 1:
                        nc.scalar.copy(ys[:], py[:])
                    else:
                        nc.vector.tensor_copy(ys[:], py[:])
                    nc.sync.dma_start(
                        y[128 * st:128 * (st + 1), 512 * nk:512 * (nk + 1)], ys[:]
                    )

            def proj(jq, copy_eng=None):
                for st4 in range(4):
                    proj_st(jq, st4, copy_eng)

            # tail variant: both ct=0 (pair 0, ready early) matmuls first so
            # they run while the pair-1 norm finishes, then both ct=1
            def proj_tail_st(jq, st4):
                st = 4 * jq + st4
                pys = []
                for nk in range(2):
                    py = psO.tile([128, 512], F32, tag="po", bufs=2, name="py")
                    nc.tensor.matmul(
                        py[:], otc[jq][0][:, 128 * st4:128 * (st4 + 1)],
                        wpt[:, 0, 512 * nk:512 * (nk + 1)],
                        start=True, stop=False, skip_group_check=True)
                    pys.append(py)
                for nk in range(2):
                    nc.tensor.matmul(
                        pys[nk][:], otc[jq][1][:, 128 * st4:128 * (st4 + 1)],
                        wpt[:, 1, 512 * nk:512 * (nk + 1)],
                        start=False, stop=True, skip_group_check=True)
                for nk in range(2):
                    ys = ysbp.tile([128, 512], BF16)
                    if nk == 1:
                        nc.scalar.copy(ys[:], pys[nk][:])
                    else:
                        nc.vector.tensor_copy(ys[:], pys[nk][:])
                    nc.sync.dma_start(
                        y[128 * st:128 * (st + 1), 512 * nk:512 * (nk + 1)], ys[:]
                    )

            # Interleave QKV chunks and proj between attention sub-phases:
            # attention is ACT(exp)-bound, so the PE queue gets dense
            # independent matmul work to stay HAM-warm, and proj trails one
            # sub-phase behind so its ot deps (incl. the head-B shift DMA)
            # are long complete when the PE reaches it.
            def attn_full(pr, jq, mids, last_fill=None, tail_norm=False):
                """attention with qkv/proj work interleaved at explicit
                ik split points: mids = [(split_ik, fn), ...] ascending"""
                nik = 4 * jq + 4
                o_ab = [psO.tile([128, 512], F32, tag="oab", bufs=2, name="o_ab")
                        for _ in range(2)]
                prev = 0
                for split, fn in mids:
                    attn_part(pr, jq, prev, split, o_ab)
                    fn()
                    prev = split
                attn_part(pr, jq, prev, nik, o_ab, last_fill=last_fill)
                if tail_norm:
                    return o_ab
                attn_norm(pr, jq, o_ab)

            qk_pair(0, 0)
            v_chunk(0)
            attn_full(0, 0, [(2, lambda: qk_half(0, 1, 0))],
                      last_fill=lambda: qk_half(0, 1, 1))
            attn_full(1, 0, [(1, lambda: qk_half(1, 0, 0)),
                             (2, lambda: qk_half(1, 0, 1)),
                             (3, lambda: qk_half(1, 1, 0))],
                      last_fill=lambda: qk_half(1, 1, 1))
            attn_full(0, 1, [(4, lambda: v_st(1, 0)),
                             (5, lambda: v_st(1, 1)),
                             (6, lambda: v_st(1, 2))],
                      last_fill=lambda: v_st(1, 3))
            attn_full(1, 1, [(2, lambda: proj_st(0, 0)),
                             (4, lambda: qk_half(2, 0, 0)),
                             (5, lambda: qk_half(2, 0, 1)),
                             (6, lambda: proj_st(0, 1)),
                             (7, lambda: proj_st(0, 2))],
                      last_fill=lambda: proj_st(0, 3))
            attn_full(0, 2, [(4, lambda: qk_half(2, 1, 0)),
                             (6, lambda: qk_half(2, 1, 1)),
                             (8, lambda: v_st(2, 0)),
                             (9, lambda: v_st(2, 1)),
                             (10, lambda: v_st(2, 2))],
                      last_fill=lambda: v_st(2, 3))
            attn_full(1, 2, [(3, lambda: proj_st(1, 0)),
                             (5, lambda: proj_st(1, 1)),
                             (6, lambda: qk_half(3, 0, 0)),
                             (8, lambda: qk_half(3, 0, 1)),
                             (10, lambda: proj_st(1, 2))],
                      last_fill=lambda: proj_st(1, 3))
            attn_full(0, 3, [(6, lambda: qk_half(3, 1, 0)),
                             (8, lambda: qk_half(3, 1, 1)),
                             (10, lambda: v_st(3, 0)),
                             (12, lambda: v_st(3, 1)),
                             (14, lambda: v_st(3, 2))],
                      last_fill=lambda: v_st(3, 3))
            o_ab_last = attn_full(1, 3, [(4, lambda: proj_st(2, 0)),
                                         (8, lambda: proj_st(2, 1)),
                                         (12, lambda: proj_st(2, 2))],
                                  last_fill=lambda: proj_st(2, 3),
                                  tail_norm=True)
            # tail: start the reciprocal chains (no PE), let the first
            # s-tile's pair-0 proj matmuls keep the PE busy meanwhile, then
            # broadcast+normalize on a free PE, then finish the projection
            pre = attn_norm_tail_pre(o_ab_last)
            pys0 = []
            for nk in range(2):
                py = psO.tile([128, 512], F32, tag="po", bufs=2, name="py")
                nc.tensor.matmul(
                    py[:], otc[3][0][:, 0:128],
                    wpt[:, 0, 512 * nk:512 * (nk + 1)],
                    start=True, stop=False, skip_group_check=True)
                pys0.append(py)
            attn_norm_tail_post(1, 3, pre)
            for nk in range(2):
                nc.tensor.matmul(
                    pys0[nk][:], otc[3][1][:, 0:128],
                    wpt[:, 1, 512 * nk:512 * (nk + 1)],
                    start=False, stop=True, skip_group_check=True)
            for nk in range(2):
                ys = ysbp.tile([128, 512], BF16)
                if nk == 1:
                    nc.scalar.copy(ys[:], pys0[nk][:])
                else:
                    nc.vector.tensor_copy(ys[:], pys0[nk][:])
                nc.sync.dma_start(
                    y[128 * 12:128 * 13, 512 * nk:512 * (nk + 1)], ys[:])
            for st4 in range(1, 4):
                proj_tail_st(3, st4)

    nc.compile()
    return nc


def _get_nc():
    global _CACHED_NC
    if _CACHED_NC is None:
        _CACHED_NC = _build()
    return _CACHED_NC


def _diag_masks() -> np.ndarray:
    return np.ascontiguousarray(
        np.tril(np.ones((128, 128), dtype=np.float32)).T
    ).astype(ml_dtypes.bfloat16)


def _in_maps(x, W_qkv, W_proj):
    masks = _diag_masks()
    maps = []
    for c in range(N_CORES):
        b, g = divmod(c, 4)
        xT = np.ascontiguousarray(x[b].T).astype(ml_dtypes.bfloat16)
        wq = W_qkv[:, GC * g:GC * (g + 1)]
        wk = W_qkv[:, E + GC * g:E + GC * (g + 1)]
        wv = W_qkv[:, 2 * E + GC * g:2 * E + GC * (g + 1)]
        w = np.ascontiguousarray(
            np.concatenate([wq, wk, wv], axis=1)).astype(ml_dtypes.bfloat16)
        wp = np.ascontiguousarray(
            W_proj[GC * g:GC * (g + 1), :]).astype(ml_dtypes.bfloat16)
        maps.append({"xT": xT, "w": w, "wp": wp, "mask": masks})
    return maps


def _run(x, W_qkv, W_proj, trace=False, **spmd_kwargs):
    nc = _get_nc()
    res = run_bass_kernel_spmd(nc, _in_maps(x, W_qkv, W_proj),
                               list(range(N_CORES)), trace=trace, **spmd_kwargs)
    out = np.zeros((B, S, E), dtype=np.float32)
    for c in range(N_CORES):
        out[c // 4] += res.results[c]["y"].astype(np.float32)
    return out, res


def kernel(x, attention_mask, W_qkv, W_proj):
    x = np.asarray(x, dtype=np.float32)
    W_qkv = np.asarray(W_qkv, dtype=np.float32)
    W_proj = np.asarray(W_proj, dtype=np.float32)
    out, _ = _run(x, W_qkv, W_proj, trace=False)
    return out
